# revision 13
# baseline (speedup 1.0000x reference)
"""Trainium2 Bass kernel for nn_CTRule (temporal KG scoring model).

Computes, for each of B=1024 queries (h, r, t):
  v = f(E0[h], E1[r], time tables, rule tables)   # [B, 128] elementwise algebra
  scores = v @ E0.T                               # [B, 40000]

Distribution over the 8 NeuronCores: 2-way batch x 4-way entity grid.
Core c handles batch rows [bh*512, bh*512+512) (bh = c//4) against entity
columns [es*10000, es*10000+10000) (es = c%4).  Per-core HBM traffic:
  out 10.24 MB + E0T slice 2.56 MB + tables ~1.2 MB  ->  ~39 us at the
358 GB/s per-core HBM limit, which (plus the ~8 us engine preamble) is the
kernel's floor.

Host prep: per-example table rows are pre-indexed on the host into one TBL
tensor ([128, 4 tiles, 1152] per core) laid out in the block patterns the
head algebra wants, so every complex/quaternion product is one wide fp16
multiply followed by a 128-wide "fold" add/sub:
  cmul(x, y)        = fold(+) of  [x0|x0|x1|x1] * [y0|y1|-y1|y0]
  complex_mul(x, y) = fold(+) of  [x0|x0|x1|-x1] * [y0|y1|y1|y0]
  mul4 tail         = fold(-/+) of Y * X1 and rev64(Y) * X1
has_rules / rule_S enter as per-partition f32 scalars (tensor_scalar).

Schedule (all engines near-saturated):
  * ALL input DMAs go on the sync HWDGE ring in dependency-latency order
    (tbl0, hrs, ident, e0t chunk0, tbl1, tbl2, tbl3, e0t bulk) — the two
    HWDGE rings share the 16 SDMA engines, so a second ring's bulk loads
    would delay the latency-critical table loads.  OUT chunks follow on
    the same FIFO; the ring never idles.
  * Heads: Vector computes tiles 0,1; GpSimd (slow but otherwise idle)
    computes tiles 2,3 concurrently.  v transposes on TensorE.
  * Scores: 512-col matmul chunks (PSUM-bank aligned!) into [P,1024] f32
    PSUM groups; groups drain via f32->fp16 casts alternating Scalar /
    Vector (GPSIMD cannot read PSUM); every 2 groups one [128,2048] OUT
    chunk is queued on the sync ring.
No cross-core communication; the host reassembles the 8 blocks.
"""

import numpy as np

P = 128
B = 1024
RANK = 128
NENT = 40000
NTIME = 365
CYCLE = 120
NCORES = 8
ES = 4                   # entity-axis splits
BS = 2                   # batch-axis splits
NSLICE = NENT // ES      # 10000 entity columns per core
ROWS = B // BS           # 512 rows per core
NT = ROWS // P           # 4 batch tiles per core
TW = 1152                # table width per tile (see column map below)
# matmul/cast groups: [P,1024] f32 = 2 PSUM banks; chunks must be 512-col
# bank-aligned (a 500-col chunk crossing a bank boundary corrupts results).
GROUPS = [(c, 1024) for c in range(0, 9216, 1024)] + [(9216, 784)]
GRP = 1024               # first E0T chunk width

# TBL column map (per tile):
C_RELX4 = 0      # [R0|R0|R1|-R1]           256
C_RCP = 256      # [RC0|RC1|-RC1|RC0]       256
C_CT = 512       # comp_time = E4[t]        128
C_TM = 640       # time = E2[t]+E5[tb]      128
C_TE = 768       # time_ent = E3[t]+E6[tb]  128
C_E0G = 896      # [L0|L1|-L1|-L0]          256

TRACE = False            # set by test harness for profiling runs
_CACHE = {}


def _build():
    import concourse.bass as bass
    import concourse.mybir as mybir
    import concourse.tile as tile
    from concourse import bacc

    dt = mybir.dt
    mult = mybir.AluOpType.mult
    add = mybir.AluOpType.add
    sub = mybir.AluOpType.subtract

    nc = bacc.Bacc("TRN2", target_bir_lowering=False, debug=False,
                   num_devices=NCORES)

    TBL = nc.dram_tensor("TBL", [P, NT, TW], dt.float16, kind="ExternalInput").ap()
    HRS = nc.dram_tensor("HRS", [P, NT, 2], dt.float32, kind="ExternalInput").ap()
    E0T = nc.dram_tensor("E0T", [RANK, NSLICE], dt.float16, kind="ExternalInput").ap()
    IDN = nc.dram_tensor("IDN", [P, P], dt.float16, kind="ExternalInput").ap()
    OUT = nc.dram_tensor("OUT", [ROWS, NSLICE], dt.float16, kind="ExternalOutput").ap()

    def r4(ap):
        # view last dim as 4 blocks of 64
        return ap.rearrange("p t (s x) -> p t s x", s=4)

    def r2(ap):
        return ap.rearrange("p t (s x) -> p t s x", s=2)

    with tile.TileContext(nc) as tc:
        with (
            tc.tile_pool(name="const", bufs=1) as constp,
            tc.tile_pool(name="ew", bufs=1) as ew,
            tc.tile_pool(name="pst", bufs=1, space="PSUM") as pst,
            tc.tile_pool(name="psm", bufs=3, space="PSUM") as psm,
        ):
            tblt = [constp.tile([P, 1, TW], dt.float16, name=f"tbl{j}")
                    for j in range(NT)]
            hrs = constp.tile([P, NT, 2], dt.float32, name="hrs")
            e0t = constp.tile([RANK, NSLICE], dt.float16)
            ident = constp.tile([P, P], dt.float16)
            nc.sync.dma_start(tblt[0][:], TBL[:, 0:1, :])
            nc.sync.dma_start(hrs[:], HRS[:])
            nc.sync.dma_start(ident[:], IDN[:])
            nc.sync.dma_start(e0t[:, 0:GRP], E0T[:, 0:GRP])
            nc.sync.dma_start(tblt[1][:], TBL[:, 1:2, :])
            nc.sync.dma_start(tblt[2][:], TBL[:, 2:3, :])
            nc.sync.dma_start(tblt[3][:], TBL[:, 3:4, :])
            nc.sync.dma_start(e0t[:, GRP:4096], E0T[:, GRP:4096])
            nc.sync.dma_start(e0t[:, 4096:7168], E0T[:, 4096:7168])
            nc.sync.dma_start(e0t[:, 7168:NSLICE], E0T[:, 7168:NSLICE])

            # ---- head: ~26 wide fp16 ops per tile (VectorE or GpSimd)
            def head(j, eng):
                t = tblt[j][:]               # [P, 1, TW]
                T = lambda a, b: t[:, :, a:b]
                nt = 1
                mk = lambda w, n: ew.tile([P, nt, w], dt.float16, name=f"{n}{j}")
                ctd = mk(256, 'ctd')
                pa, pb, pc = mk(256, 'pa'), mk(256, 'pb'), mk(256, 'pc')
                fa, bt, bc = mk(128, 'fa'), mk(128, 'bt'), mk(128, 'bc')
                g, hsr = mk(128, 'g'), mk(128, 'hsr')
                w2, fc = mk(256, 'w2'), mk(128, 'fc')
                yy, x1 = mk(256, 'yy'), mk(256, 'x1')
                ma, mb = mk(256, 'ma'), mk(256, 'mb')
                fm, fn, vv = mk(128, 'fm'), mk(128, 'fn'), mk(128, 'vv')

                def TT(out, a, b, op):
                    eng.tensor_tensor(out=out, in0=a, in1=b, op=op)

                # CTdup = [C0|C0|C1|C1] built on-chip (2 strided copies)
                eng.tensor_copy(out=r4(ctd[:])[:, :, 0::2, :],
                                in_=r2(T(C_CT, C_CT + 128)))
                eng.tensor_copy(out=r4(ctd[:])[:, :, 1::2, :],
                                in_=r2(T(C_CT, C_CT + 128)))
                # rule branch: fa = cmul(CT, RC)
                TT(pa[:], ctd[:], T(C_RCP, C_RCP + 256), mult)
                TT(fa[:], pa[:, :, 0:128], pa[:, :, 128:256], add)
                # no-rule branch: bt = lhs + cmul(rel, lhs)
                TT(pb[:], T(C_RELX4, C_RELX4 + 256), T(C_E0G, C_E0G + 256), mult)
                TT(bt[:], pb[:, :, 0:128], pb[:, :, 128:256], add)
                TT(bt[:], bt[:], T(C_E0G, C_E0G + 128), add)
                # bc = bt + CT
                TT(bc[:], bt[:], T(C_CT, C_CT + 128), add)
                # w = hr*(fa - bt) - hr*rS*rel + bt + CT
                TT(g[:], fa[:], bt[:], sub)
                eng.tensor_scalar(out=g[:], in0=g[:],
                                  scalar1=hrs[:, j:j + 1, 0:1],
                                  scalar2=None, op0=mult)
                eng.tensor_scalar(out=r2(hsr[:]),
                                  in0=r4(T(C_RELX4, C_RELX4 + 256))[:, :, 0::2, :],
                                  scalar1=hrs[:, j:j + 1, 1:2],
                                  scalar2=None, op0=mult)
                TT(g[:], g[:], hsr[:], sub)
                TT(w2[:, :, 0:128], g[:], bc[:], add)
                eng.tensor_copy(out=r2(w2[:, :, 128:256]),
                                in_=r2(w2[:, :, 0:128])[:, :, ::-1, :])
                # rel_ = rel + complex_mul(rel, w) -> Y blocks {0,2}
                TT(pc[:], T(C_RELX4, C_RELX4 + 256), w2[:], mult)
                TT(fc[:], pc[:, :, 0:128], pc[:, :, 128:256], add)
                TT(r4(yy[:])[:, :, 0::2, :], r2(fc[:]),
                   r4(T(C_RELX4, C_RELX4 + 256))[:, :, 0::2, :], add)
                # Y blocks {1,3} = TM halves
                eng.tensor_copy(out=r4(yy[:])[:, :, 1::2, :],
                                in_=r2(T(C_TM, C_TM + 128)))
                # X1 = [L0+T0 | L0-T0 | L1-T1 | L1+T1]
                TT(r4(x1[:])[:, :, 0::3, :], r2(T(C_E0G, C_E0G + 128)),
                   r2(T(C_TE, C_TE + 128)), add)
                TT(r4(x1[:])[:, :, 1:3, :], r2(T(C_E0G, C_E0G + 128)),
                   r2(T(C_TE, C_TE + 128)), sub)
                # v
                TT(ma[:], yy[:], x1[:], mult)
                TT(mb[:], r4(yy[:])[:, :, ::-1, :], x1[:], mult)
                TT(fm[:], ma[:, :, 0:128], ma[:, :, 128:256], sub)
                TT(vv[:, :, 0:64], fm[:, :, 0:64], fm[:, :, 64:128], add)
                TT(fn[:], mb[:, :, 0:128], mb[:, :, 128:256], add)
                TT(vv[:, :, 64:128], fn[:, :, 0:64], fn[:, :, 64:128], add)
                return vv

            v0 = head(0, nc.vector)
            # tiles 2,3 on GpSimd: ~12 us per tile but fully parallel; their
            # vts aren't needed until the output stream reaches tiles 2/3.
            v2 = head(2, nc.gpsimd)
            vts = {}

            def finish_vt(vsrc, j):
                vt_ps = pst.tile([P, P], dt.float16, space="PSUM", tag="vtps")
                nc.tensor.transpose(out=vt_ps[:], in_=vsrc[:, 0, :],
                                    identity=ident[:])
                vt = constp.tile([P, P], dt.float16, name=f"vt{j}")
                nc.scalar.copy(out=vt[:], in_=vt_ps[:])
                vts[j] = vt

            finish_vt(v0, 0)
            v1 = head(1, nc.vector)
            v3 = head(3, nc.gpsimd)

            osb = [constp.tile([P, NSLICE], dt.float16, name=f"osb{i}")
                   for i in range(NT)]
            cast_cnt = [0]

            def cast(dst, src, engine):
                # GPSIMD cannot read PSUM; only Scalar(ACT)/Vector can drain
                if engine == 0:
                    nc.scalar.copy(out=dst, in_=src)
                else:
                    nc.vector.tensor_copy(out=dst, in_=src)

            for j in range(NT):
                if j == 1:
                    finish_vt(v1, 1)
                    finish_vt(v2, 2)
                    finish_vt(v3, 3)
                ob = osb[j]
                for gi, (c0, gw) in enumerate(GROUPS):
                    mm = psm.tile([P, 1024], dt.float32, space="PSUM", tag="mm")
                    for lo in range(0, gw, 512):
                        cw = min(512, gw - lo)
                        nc.tensor.matmul(out=mm[:, lo:lo + cw],
                                         lhsT=vts[j][:],
                                         rhs=e0t[:, c0 + lo:c0 + lo + cw],
                                         start=True, stop=True)
                    eng = cast_cnt[0] % 2   # alternate scalar/vector
                    cast_cnt[0] += 1
                    cast(ob[:, c0:c0 + gw], mm[:, 0:gw], eng)
                    if gi % 2 == 1:
                        oc, ow = GROUPS[gi - 1][0], GROUPS[gi - 1][1] + gw
                        nc.sync.dma_start(
                            OUT[j * P:(j + 1) * P, oc:oc + ow],
                            ob[:, oc:oc + ow])

    nc.compile()
    return nc


def _prep_inputs(inputs):
    x = np.asarray(inputs["x"])
    E0 = np.asarray(inputs["E0"], dtype=np.float32)
    E1 = np.asarray(inputs["E1"], dtype=np.float32)
    E2 = np.asarray(inputs["E2"], dtype=np.float32)
    E3 = np.asarray(inputs["E3"], dtype=np.float32)
    E4 = np.asarray(inputs["E4"], dtype=np.float32)
    E5 = np.asarray(inputs["E5"], dtype=np.float32)
    E6 = np.asarray(inputs["E6"], dtype=np.float32)
    rule_C = np.asarray(inputs["rule_C"], dtype=np.float32)
    rule_S = np.asarray(inputs["rule_S"], dtype=np.float32)
    has_rules = np.asarray(inputs["has_rules"])

    h, r, t = (x[:, 0].astype(np.int64), x[:, 1].astype(np.int64),
               x[:, 3].astype(np.int64))
    tb = t // CYCLE
    H = RANK // 2

    L = E0[h]
    R = E1[r]
    RC = rule_C[r]
    CT = E4[t]
    TM = E2[t] + E5[tb]
    TE = E3[t] + E6[tb]
    hr = has_rules[r].astype(np.float32)
    hs = hr * rule_S[r]

    def hsp(a):
        return a[:, :H], a[:, H:]

    L0, L1 = hsp(L)
    R0, R1 = hsp(R)
    RC0, RC1 = hsp(RC)

    tblex = np.concatenate([
        R0, R0, R1, -R1,          # RELX4
        RC0, RC1, -RC1, RC0,      # RCP
        CT, TM, TE,
        L0, L1, -L1, -L0,         # E0GX
    ], axis=1).astype(np.float16)   # [B, TW]
    assert tblex.shape[1] == TW
    hrsex = np.stack([hr, hs], axis=1).astype(np.float32)   # [B, 2]

    e0t = np.ascontiguousarray(E0.T.astype(np.float16))   # [128, 40000]

    tbl_by_bh, hrs_by_bh = [], []
    for bh in range(BS):
        rows = tblex[bh * ROWS:(bh + 1) * ROWS]
        tbl_by_bh.append(np.ascontiguousarray(
            rows.reshape(NT, P, TW).transpose(1, 0, 2)))
        hrows = hrsex[bh * ROWS:(bh + 1) * ROWS]
        hrs_by_bh.append(np.ascontiguousarray(
            hrows.reshape(NT, P, 2).transpose(1, 0, 2)))
    e0t_by_es = [np.ascontiguousarray(e0t[:, es * NSLICE:(es + 1) * NSLICE])
                 for es in range(ES)]

    ident = np.eye(P, dtype=np.float16)
    in_maps = []
    for c in range(NCORES):
        in_maps.append({
            "TBL": tbl_by_bh[c // ES],
            "HRS": hrs_by_bh[c // ES],
            "E0T": e0t_by_es[c % ES],
            "IDN": ident,
        })
    return in_maps


def kernel(**inputs):
    from concourse.bass_utils import run_bass_kernel_spmd

    if "nc" not in _CACHE:
        _CACHE["nc"] = _build()
    nc = _CACHE["nc"]

    in_maps = _prep_inputs(inputs)
    res = run_bass_kernel_spmd(nc, in_maps, core_ids=list(range(NCORES)),
                               trace=TRACE)
    _CACHE["last_result"] = res
    out = np.empty((B, NENT), np.float32)
    for c in range(NCORES):
        bh, es = c // ES, c % ES
        out[bh * ROWS:(bh + 1) * ROWS,
            es * NSLICE:(es + 1) * NSLICE] = res.results[c]["OUT"]
    return out


# revision 14
# speedup vs baseline: 1.1543x; 1.1543x over previous
"""Trainium2 Bass kernel for nn_CTRule (temporal KG scoring model).

Computes, for each of B=1024 queries (h, r, t):
  v = f(E0[h], E1[r], time tables, rule tables)   # [B, 128] elementwise algebra
  scores = v @ E0.T                               # [B, 40000]

Distribution over the 8 NeuronCores: 2-way batch x 4-way entity grid.
Core c handles batch rows [bh*512, bh*512+512) (bh = c//4) against entity
columns [es*10000, es*10000+10000) (es = c%4).  Per-core HBM traffic:
  out 10.24 MB + E0T slice 2.56 MB + tables ~1.2 MB  ->  ~39 us at the
358 GB/s per-core HBM limit, which (plus the ~8 us engine preamble) is the
kernel's floor.

Host prep: per-example table rows are pre-indexed on the host into one TBL
tensor ([128, 4 tiles, 1152] per core) laid out in the block patterns the
head algebra wants, so every complex/quaternion product is one wide fp16
multiply followed by a 128-wide "fold" add/sub:
  cmul(x, y)        = fold(+) of  [x0|x0|x1|x1] * [y0|y1|-y1|y0]
  complex_mul(x, y) = fold(+) of  [x0|x0|x1|-x1] * [y0|y1|y1|y0]
  mul4 tail         = fold(-/+) of Y * X1 and rev64(Y) * X1
has_rules / rule_S enter as per-partition f32 scalars (tensor_scalar).

Schedule (all engines near-saturated):
  * ALL input DMAs go on the sync HWDGE ring in dependency-latency order
    (tbl0, hrs, ident, e0t chunk0, tbl1, tbl2, tbl3, e0t bulk) — the two
    HWDGE rings share the 16 SDMA engines, so a second ring's bulk loads
    would delay the latency-critical table loads.  OUT chunks follow on
    the same FIFO; the ring never idles.
  * Heads: Vector computes tiles 0,1; GpSimd (slow but otherwise idle)
    computes tiles 2,3 concurrently.  v transposes on TensorE.
  * Scores: 512-col matmul chunks (PSUM-bank aligned!) into [P,1024] f32
    PSUM groups; groups drain via f32->fp16 casts alternating Scalar /
    Vector (GPSIMD cannot read PSUM); every 2 groups one [128,2048] OUT
    chunk is queued on the sync ring.
No cross-core communication; the host reassembles the 8 blocks.
"""

import numpy as np

P = 128
B = 1024
RANK = 128
NENT = 40000
NTIME = 365
CYCLE = 120
NCORES = 8
ES = 4                   # entity-axis splits
BS = 2                   # batch-axis splits
NSLICE = NENT // ES      # 10000 entity columns per core
ROWS = B // BS           # 512 rows per core
NT = ROWS // P           # 4 batch tiles per core
TW = 1536                # table width per tile (see column map below)
# matmul/cast groups: [P,1024] f32 = 2 PSUM banks; chunks must be 512-col
# bank-aligned (a 500-col chunk crossing a bank boundary corrupts results).
GROUPS = [(c, 1024) for c in range(0, 9216, 1024)] + [(9216, 784)]
GRP = 1024               # first E0T chunk width

# TBL column map (per tile):
C_RELX4 = 0      # [R0|R0|R1|-R1]           256
C_RCP = 256      # [RC0|RC1|-RC1|RC0]       256
C_CTD = 512      # [C0|C0|C1|C1] (CT dup)   256
C_TM = 768       # time = E2[t]+E5[tb]      128
C_TE = 896       # time_ent = E3[t]+E6[tb]  128
C_E0G = 1024     # [L0|L1|-L1|-L0]          256
C_HRW = 1280     # has_rules broadcast      128
C_HSR = 1408     # hr*rS*rel                128

TRACE = False            # set by test harness for profiling runs
_CACHE = {}


def _build():
    import concourse.bass as bass
    import concourse.mybir as mybir
    import concourse.tile as tile
    from concourse import bacc

    dt = mybir.dt
    mult = mybir.AluOpType.mult
    add = mybir.AluOpType.add
    sub = mybir.AluOpType.subtract

    nc = bacc.Bacc("TRN2", target_bir_lowering=False, debug=False,
                   num_devices=NCORES)

    TBL = nc.dram_tensor("TBL", [P, NT, TW], dt.float16, kind="ExternalInput").ap()
    E0T = nc.dram_tensor("E0T", [RANK, NSLICE], dt.float16, kind="ExternalInput").ap()
    IDN = nc.dram_tensor("IDN", [P, P], dt.float16, kind="ExternalInput").ap()
    OUT = nc.dram_tensor("OUT", [ROWS, NSLICE], dt.float16, kind="ExternalOutput").ap()

    def r4(ap):
        # view last dim as 4 blocks of 64
        return ap.rearrange("p t (s x) -> p t s x", s=4)

    def r2(ap):
        return ap.rearrange("p t (s x) -> p t s x", s=2)

    with tile.TileContext(nc) as tc:
        with (
            tc.tile_pool(name="const", bufs=1) as constp,
            tc.tile_pool(name="ew", bufs=1) as ew,
            tc.tile_pool(name="pst", bufs=1, space="PSUM") as pst,
            tc.tile_pool(name="psm", bufs=3, space="PSUM") as psm,
        ):
            tblt = [constp.tile([P, 1, TW], dt.float16, name=f"tbl{j}")
                    for j in range(NT)]
            e0t = constp.tile([RANK, NSLICE], dt.float16)
            ident = constp.tile([P, P], dt.float16)
            nc.sync.dma_start(tblt[0][:], TBL[:, 0:1, :])
            nc.sync.dma_start(ident[:], IDN[:])
            nc.sync.dma_start(e0t[:, 0:GRP], E0T[:, 0:GRP])
            nc.sync.dma_start(tblt[1][:], TBL[:, 1:2, :])
            nc.sync.dma_start(tblt[2][:], TBL[:, 2:3, :])
            nc.sync.dma_start(tblt[3][:], TBL[:, 3:4, :])
            nc.sync.dma_start(e0t[:, GRP:4096], E0T[:, GRP:4096])
            nc.sync.dma_start(e0t[:, 4096:7168], E0T[:, 4096:7168])
            nc.sync.dma_start(e0t[:, 7168:NSLICE], E0T[:, 7168:NSLICE])

            # ---- head: ~26 wide fp16 ops per tile (VectorE or GpSimd)
            def head(j, eng):
                t = tblt[j][:]               # [P, 1, TW]
                T = lambda a, b: t[:, :, a:b]
                nt = 1
                mk = lambda w, n: ew.tile([P, nt, w], dt.float16, name=f"{n}{j}")
                pa, pb, pc = mk(256, 'pa'), mk(256, 'pb'), mk(256, 'pc')
                fa, bt, bc = mk(128, 'fa'), mk(128, 'bt'), mk(128, 'bc')
                g = mk(128, 'g')
                w2, fc = mk(256, 'w2'), mk(128, 'fc')
                yy, x1 = mk(256, 'yy'), mk(256, 'x1')
                ma, mb = mk(256, 'ma'), mk(256, 'mb')
                fm, fn, vv = mk(128, 'fm'), mk(128, 'fn'), mk(128, 'vv')

                def TT(out, a, b, op):
                    eng.tensor_tensor(out=out, in0=a, in1=b, op=op)

                # rule branch: fa = cmul(CT, RC)
                TT(pa[:], T(C_CTD, C_CTD + 256), T(C_RCP, C_RCP + 256), mult)
                TT(fa[:], pa[:, :, 0:128], pa[:, :, 128:256], add)
                # no-rule branch: bt = lhs + cmul(rel, lhs)
                TT(pb[:], T(C_RELX4, C_RELX4 + 256), T(C_E0G, C_E0G + 256), mult)
                TT(bt[:], pb[:, :, 0:128], pb[:, :, 128:256], add)
                TT(bt[:], bt[:], T(C_E0G, C_E0G + 128), add)
                # bc = bt + CT (CT = blocks {0,2} of CTdup)
                TT(r2(bc[:]), r2(bt[:]),
                   r4(T(C_CTD, C_CTD + 256))[:, :, 0::2, :], add)
                # w = hr*(fa - bt) - hr*rS*rel + bt + CT
                TT(g[:], fa[:], bt[:], sub)
                TT(g[:], g[:], T(C_HRW, C_HRW + 128), mult)
                TT(g[:], g[:], T(C_HSR, C_HSR + 128), sub)
                TT(w2[:, :, 0:128], g[:], bc[:], add)
                eng.tensor_copy(out=r2(w2[:, :, 128:256]),
                                in_=r2(w2[:, :, 0:128])[:, :, ::-1, :])
                # rel_ = rel + complex_mul(rel, w) -> Y blocks {0,2}
                TT(pc[:], T(C_RELX4, C_RELX4 + 256), w2[:], mult)
                TT(fc[:], pc[:, :, 0:128], pc[:, :, 128:256], add)
                TT(r4(yy[:])[:, :, 0::2, :], r2(fc[:]),
                   r4(T(C_RELX4, C_RELX4 + 256))[:, :, 0::2, :], add)
                # Y blocks {1,3} = TM halves
                eng.tensor_copy(out=r4(yy[:])[:, :, 1::2, :],
                                in_=r2(T(C_TM, C_TM + 128)))
                # X1 = [L0+T0 | L0-T0 | L1-T1 | L1+T1]
                TT(r4(x1[:])[:, :, 0::3, :], r2(T(C_E0G, C_E0G + 128)),
                   r2(T(C_TE, C_TE + 128)), add)
                TT(r4(x1[:])[:, :, 1:3, :], r2(T(C_E0G, C_E0G + 128)),
                   r2(T(C_TE, C_TE + 128)), sub)
                # v
                TT(ma[:], yy[:], x1[:], mult)
                TT(mb[:], r4(yy[:])[:, :, ::-1, :], x1[:], mult)
                TT(fm[:], ma[:, :, 0:128], ma[:, :, 128:256], sub)
                TT(vv[:, :, 0:64], fm[:, :, 0:64], fm[:, :, 64:128], add)
                TT(fn[:], mb[:, :, 0:128], mb[:, :, 128:256], add)
                TT(vv[:, :, 64:128], fn[:, :, 0:64], fn[:, :, 64:128], add)
                return vv

            v0 = head(0, nc.vector)
            # tiles 2,3 on GpSimd: ~12 us per tile but fully parallel; their
            # vts aren't needed until the output stream reaches tiles 2/3.
            v2 = head(2, nc.gpsimd)
            vts = {}

            def finish_vt(vsrc, j):
                vt_ps = pst.tile([P, P], dt.float16, space="PSUM", tag="vtps")
                nc.tensor.transpose(out=vt_ps[:], in_=vsrc[:, 0, :],
                                    identity=ident[:])
                vt = constp.tile([P, P], dt.float16, name=f"vt{j}")
                nc.scalar.copy(out=vt[:], in_=vt_ps[:])
                vts[j] = vt

            finish_vt(v0, 0)
            v1 = head(1, nc.vector)
            v3 = head(3, nc.gpsimd)

            osb = [constp.tile([P, NSLICE], dt.float16, name=f"osb{i}")
                   for i in range(NT)]
            cast_cnt = [0]

            def cast(dst, src, engine):
                # GPSIMD cannot read PSUM; only Scalar(ACT)/Vector can drain
                if engine == 0:
                    nc.scalar.copy(out=dst, in_=src)
                else:
                    nc.vector.tensor_copy(out=dst, in_=src)

            for j in range(NT):
                if j == 1:
                    finish_vt(v1, 1)
                    finish_vt(v2, 2)
                    finish_vt(v3, 3)
                ob = osb[j]
                for gi, (c0, gw) in enumerate(GROUPS):
                    mm = psm.tile([P, 1024], dt.float32, space="PSUM", tag="mm")
                    for lo in range(0, gw, 512):
                        cw = min(512, gw - lo)
                        nc.tensor.matmul(out=mm[:, lo:lo + cw],
                                         lhsT=vts[j][:],
                                         rhs=e0t[:, c0 + lo:c0 + lo + cw],
                                         start=True, stop=True)
                    eng = cast_cnt[0] % 2   # alternate scalar/vector
                    cast_cnt[0] += 1
                    cast(ob[:, c0:c0 + gw], mm[:, 0:gw], eng)
                    if gi % 2 == 1:
                        oc, ow = GROUPS[gi - 1][0], GROUPS[gi - 1][1] + gw
                        nc.sync.dma_start(
                            OUT[j * P:(j + 1) * P, oc:oc + ow],
                            ob[:, oc:oc + ow])

    nc.compile()
    return nc


def _prep_inputs(inputs):
    x = np.asarray(inputs["x"])
    E0 = np.asarray(inputs["E0"], dtype=np.float32)
    E1 = np.asarray(inputs["E1"], dtype=np.float32)
    E2 = np.asarray(inputs["E2"], dtype=np.float32)
    E3 = np.asarray(inputs["E3"], dtype=np.float32)
    E4 = np.asarray(inputs["E4"], dtype=np.float32)
    E5 = np.asarray(inputs["E5"], dtype=np.float32)
    E6 = np.asarray(inputs["E6"], dtype=np.float32)
    rule_C = np.asarray(inputs["rule_C"], dtype=np.float32)
    rule_S = np.asarray(inputs["rule_S"], dtype=np.float32)
    has_rules = np.asarray(inputs["has_rules"])

    h, r, t = (x[:, 0].astype(np.int64), x[:, 1].astype(np.int64),
               x[:, 3].astype(np.int64))
    tb = t // CYCLE
    H = RANK // 2

    L = E0[h]
    R = E1[r]
    RC = rule_C[r]
    CT = E4[t]
    TM = E2[t] + E5[tb]
    TE = E3[t] + E6[tb]
    hr = has_rules[r].astype(np.float32)
    hs = hr * rule_S[r]

    def hsp(a):
        return a[:, :H], a[:, H:]

    L0, L1 = hsp(L)
    R0, R1 = hsp(R)
    RC0, RC1 = hsp(RC)

    C0, C1 = hsp(CT)
    tblex = np.concatenate([
        R0, R0, R1, -R1,          # RELX4
        RC0, RC1, -RC1, RC0,      # RCP
        C0, C0, C1, C1,           # CTdup
        TM, TE,
        L0, L1, -L1, -L0,         # E0GX
        np.repeat(hr[:, None], RANK, axis=1),
        hs[:, None] * R,
    ], axis=1).astype(np.float16)   # [B, TW]
    assert tblex.shape[1] == TW

    e0t = np.ascontiguousarray(E0.T.astype(np.float16))   # [128, 40000]

    tbl_by_bh = []
    for bh in range(BS):
        rows = tblex[bh * ROWS:(bh + 1) * ROWS]
        tbl_by_bh.append(np.ascontiguousarray(
            rows.reshape(NT, P, TW).transpose(1, 0, 2)))
    e0t_by_es = [np.ascontiguousarray(e0t[:, es * NSLICE:(es + 1) * NSLICE])
                 for es in range(ES)]

    ident = np.eye(P, dtype=np.float16)
    in_maps = []
    for c in range(NCORES):
        in_maps.append({
            "TBL": tbl_by_bh[c // ES],
            "E0T": e0t_by_es[c % ES],
            "IDN": ident,
        })
    return in_maps


def kernel(**inputs):
    from concourse.bass_utils import run_bass_kernel_spmd

    if "nc" not in _CACHE:
        _CACHE["nc"] = _build()
    nc = _CACHE["nc"]

    in_maps = _prep_inputs(inputs)
    res = run_bass_kernel_spmd(nc, in_maps, core_ids=list(range(NCORES)),
                               trace=TRACE)
    _CACHE["last_result"] = res
    out = np.empty((B, NENT), np.float32)
    for c in range(NCORES):
        bh, es = c // ES, c % ES
        out[bh * ROWS:(bh + 1) * ROWS,
            es * NSLICE:(es + 1) * NSLICE] = res.results[c]["OUT"]
    return out


# revision 15
# speedup vs baseline: 1.2379x; 1.0725x over previous
"""Trainium2 Bass kernel for nn_CTRule (temporal KG scoring model).

Computes, for each of B=1024 queries (h, r, t):
  v = f(E0[h], E1[r], time tables, rule tables)   # [B, 128] elementwise algebra
  scores = v @ E0.T                               # [B, 40000]

Distribution over the 8 NeuronCores: 2-way batch x 4-way entity grid.
Core c handles batch rows [bh*512, bh*512+512) (bh = c//4) against entity
columns [es*10000, es*10000+10000) (es = c%4).  Per-core HBM traffic:
  out 10.24 MB + E0T slice 2.56 MB + tables ~1.2 MB  ->  ~39 us at the
358 GB/s per-core HBM limit, which (plus the ~8 us engine preamble) is the
kernel's floor.

Host prep: per-example table rows are pre-indexed on the host into one TBL
tensor ([128, 4 tiles, 1152] per core) laid out in the block patterns the
head algebra wants, so every complex/quaternion product is one wide fp16
multiply followed by a 128-wide "fold" add/sub:
  cmul(x, y)        = fold(+) of  [x0|x0|x1|x1] * [y0|y1|-y1|y0]
  complex_mul(x, y) = fold(+) of  [x0|x0|x1|-x1] * [y0|y1|y1|y0]
  mul4 tail         = fold(-/+) of Y * X1 and rev64(Y) * X1
has_rules / rule_S enter as per-partition f32 scalars (tensor_scalar).

Schedule (all engines near-saturated):
  * ALL input DMAs go on the sync HWDGE ring in dependency-latency order
    (tbl0, hrs, ident, e0t chunk0, tbl1, tbl2, tbl3, e0t bulk) — the two
    HWDGE rings share the 16 SDMA engines, so a second ring's bulk loads
    would delay the latency-critical table loads.  OUT chunks follow on
    the same FIFO; the ring never idles.
  * Heads: Vector computes tiles 0,1; GpSimd (slow but otherwise idle)
    computes tiles 2,3 concurrently.  v transposes on TensorE.
  * Scores: 512-col matmul chunks (PSUM-bank aligned!) into [P,1024] f32
    PSUM groups; groups drain via f32->fp16 casts alternating Scalar /
    Vector (GPSIMD cannot read PSUM); every 2 groups one [128,2048] OUT
    chunk is queued on the sync ring.
No cross-core communication; the host reassembles the 8 blocks.
"""

import numpy as np

P = 128
B = 1024
RANK = 128
NENT = 40000
NTIME = 365
CYCLE = 120
NCORES = 8
ES = 4                   # entity-axis splits
BS = 2                   # batch-axis splits
NSLICE = NENT // ES      # 10000 entity columns per core
ROWS = B // BS           # 512 rows per core
NT = ROWS // P           # 4 batch tiles per core
TW = 1536                # table width per tile (see column map below)
# matmul/cast groups: [P,1024] f32 = 2 PSUM banks; chunks must be 512-col
# bank-aligned (a 500-col chunk crossing a bank boundary corrupts results).
GROUPS = [(c, 1024) for c in range(0, 9216, 1024)] + [(9216, 784)]
GRP = 1024               # first E0T chunk width

# TBL column map (per tile):
C_RELX4 = 0      # [R0|R0|R1|-R1]           256
C_RCP = 256      # [RC0|RC1|-RC1|RC0]       256
C_CTD = 512      # [C0|C0|C1|C1] (CT dup)   256
C_TM = 768       # time = E2[t]+E5[tb]      128
C_TE = 896       # time_ent = E3[t]+E6[tb]  128
C_E0G = 1024     # [L0|L1|-L1|-L0]          256
C_HRW = 1280     # has_rules broadcast      128
C_HSR = 1408     # hr*rS*rel                128

TRACE = False            # set by test harness for profiling runs
_CACHE = {}


def _build():
    import concourse.bass as bass
    import concourse.mybir as mybir
    import concourse.tile as tile
    from concourse import bacc

    dt = mybir.dt
    mult = mybir.AluOpType.mult
    add = mybir.AluOpType.add
    sub = mybir.AluOpType.subtract

    nc = bacc.Bacc("TRN2", target_bir_lowering=False, debug=False,
                   num_devices=NCORES)

    TBL = nc.dram_tensor("TBL", [P, NT, TW], dt.float16, kind="ExternalInput").ap()
    E0T = nc.dram_tensor("E0T", [RANK, NSLICE], dt.float16, kind="ExternalInput").ap()
    IDN = nc.dram_tensor("IDN", [P, P], dt.float16, kind="ExternalInput").ap()
    OUT = nc.dram_tensor("OUT", [ROWS, NSLICE], dt.float16, kind="ExternalOutput").ap()

    def r4(ap):
        # view last dim as 4 blocks of 64
        return ap.rearrange("p t (s x) -> p t s x", s=4)

    def r2(ap):
        return ap.rearrange("p t (s x) -> p t s x", s=2)

    with tile.TileContext(nc) as tc:
        with (
            tc.tile_pool(name="const", bufs=1) as constp,
            tc.tile_pool(name="ew", bufs=1) as ew,
            tc.tile_pool(name="pst", bufs=1, space="PSUM") as pst,
            tc.tile_pool(name="psm", bufs=3, space="PSUM") as psm,
        ):
            tblt = [constp.tile([P, 1, TW], dt.float16, name=f"tbl{j}")
                    for j in range(NT)]
            e0t = constp.tile([RANK, NSLICE], dt.float16)
            ident = constp.tile([P, P], dt.float16)
            nc.sync.dma_start(tblt[0][:], TBL[:, 0:1, :])
            nc.sync.dma_start(ident[:], IDN[:])
            nc.sync.dma_start(e0t[:, 0:GRP], E0T[:, 0:GRP])
            nc.sync.dma_start(tblt[1][:], TBL[:, 1:2, :])
            nc.sync.dma_start(tblt[2][:], TBL[:, 2:3, :])
            nc.sync.dma_start(tblt[3][:], TBL[:, 3:4, :])
            nc.sync.dma_start(e0t[:, GRP:4096], E0T[:, GRP:4096])
            nc.sync.dma_start(e0t[:, 4096:7168], E0T[:, 4096:7168])
            nc.sync.dma_start(e0t[:, 7168:NSLICE], E0T[:, 7168:NSLICE])

            # ---- head: ~26 wide fp16 ops per tile (VectorE or GpSimd)
            def head(j, eng):
                t = tblt[j][:]               # [P, 1, TW]
                T = lambda a, b: t[:, :, a:b]
                nt = 1
                mk = lambda w, n: ew.tile([P, nt, w], dt.float16, name=f"{n}{j}")
                pa, pb, pc = mk(256, 'pa'), mk(256, 'pb'), mk(256, 'pc')
                fa, bt, bc = mk(128, 'fa'), mk(128, 'bt'), mk(128, 'bc')
                g = mk(128, 'g')
                w2, fc = mk(256, 'w2'), mk(128, 'fc')
                yy, x1 = mk(256, 'yy'), mk(256, 'x1')
                ma, mb = mk(256, 'ma'), mk(256, 'mb')
                fm, fn, vv = mk(128, 'fm'), mk(128, 'fn'), mk(128, 'vv')

                def TT(out, a, b, op):
                    eng.tensor_tensor(out=out, in0=a, in1=b, op=op)

                # rule branch: fa = cmul(CT, RC)
                TT(pa[:], T(C_CTD, C_CTD + 256), T(C_RCP, C_RCP + 256), mult)
                TT(fa[:], pa[:, :, 0:128], pa[:, :, 128:256], add)
                # no-rule branch: bt = lhs + cmul(rel, lhs)
                TT(pb[:], T(C_RELX4, C_RELX4 + 256), T(C_E0G, C_E0G + 256), mult)
                TT(bt[:], pb[:, :, 0:128], pb[:, :, 128:256], add)
                TT(bt[:], bt[:], T(C_E0G, C_E0G + 128), add)
                # bc = bt + CT (CT = blocks {0,2} of CTdup)
                TT(r2(bc[:]), r2(bt[:]),
                   r4(T(C_CTD, C_CTD + 256))[:, :, 0::2, :], add)
                # w = hr*(fa - bt) - hr*rS*rel + bt + CT
                TT(g[:], fa[:], bt[:], sub)
                TT(g[:], g[:], T(C_HRW, C_HRW + 128), mult)
                TT(g[:], g[:], T(C_HSR, C_HSR + 128), sub)
                TT(w2[:, :, 0:128], g[:], bc[:], add)
                eng.tensor_copy(out=r2(w2[:, :, 128:256]),
                                in_=r2(w2[:, :, 0:128])[:, :, ::-1, :])
                # rel_ = rel + complex_mul(rel, w) -> Y blocks {0,2}
                TT(pc[:], T(C_RELX4, C_RELX4 + 256), w2[:], mult)
                TT(fc[:], pc[:, :, 0:128], pc[:, :, 128:256], add)
                TT(r4(yy[:])[:, :, 0::2, :], r2(fc[:]),
                   r4(T(C_RELX4, C_RELX4 + 256))[:, :, 0::2, :], add)
                # Y blocks {1,3} = TM halves
                eng.tensor_copy(out=r4(yy[:])[:, :, 1::2, :],
                                in_=r2(T(C_TM, C_TM + 128)))
                # X1 = [L0+T0 | L0-T0 | L1-T1 | L1+T1]
                TT(r4(x1[:])[:, :, 0::3, :], r2(T(C_E0G, C_E0G + 128)),
                   r2(T(C_TE, C_TE + 128)), add)
                TT(r4(x1[:])[:, :, 1:3, :], r2(T(C_E0G, C_E0G + 128)),
                   r2(T(C_TE, C_TE + 128)), sub)
                # v
                TT(ma[:], yy[:], x1[:], mult)
                TT(mb[:], r4(yy[:])[:, :, ::-1, :], x1[:], mult)
                TT(fm[:], ma[:, :, 0:128], ma[:, :, 128:256], sub)
                TT(vv[:, :, 0:64], fm[:, :, 0:64], fm[:, :, 64:128], add)
                TT(fn[:], mb[:, :, 0:128], mb[:, :, 128:256], add)
                TT(vv[:, :, 64:128], fn[:, :, 0:64], fn[:, :, 64:128], add)
                return vv

            v0 = head(0, nc.vector)
            # tiles 2,3 on GpSimd: ~12 us per tile but fully parallel; their
            # vts aren't needed until the output stream reaches tiles 2/3.
            v2 = head(2, nc.gpsimd)
            vts = {}

            def finish_vt(vsrc, j):
                vt_ps = pst.tile([P, P], dt.float16, space="PSUM", tag="vtps")
                nc.tensor.transpose(out=vt_ps[:], in_=vsrc[:, 0, :],
                                    identity=ident[:])
                vt = constp.tile([P, P], dt.float16, name=f"vt{j}")
                nc.scalar.copy(out=vt[:], in_=vt_ps[:])
                vts[j] = vt

            finish_vt(v0, 0)
            v1 = head(1, nc.vector)
            v3 = head(3, nc.gpsimd)

            osb = [constp.tile([P, NSLICE], dt.float16, name=f"osb{i}")
                   for i in range(NT)]
            cast_cnt = [0]

            def cast(dst, src, engine):
                # GPSIMD cannot read PSUM; only Scalar(ACT)/Vector can drain
                if engine == 0:
                    nc.scalar.copy(out=dst, in_=src)
                else:
                    nc.vector.tensor_copy(out=dst, in_=src)

            vsrcs = {1: v1, 2: v2, 3: v3}
            for j in range(NT):
                if j >= 1:
                    # transpose just before this tile's matmuls: the in-order
                    # tensor engine must not park on a not-yet-ready head
                    finish_vt(vsrcs[j], j)
                ob = osb[j]
                for gi, (c0, gw) in enumerate(GROUPS):
                    mm = psm.tile([P, 1024], dt.float32, space="PSUM", tag="mm")
                    for lo in range(0, gw, 512):
                        cw = min(512, gw - lo)
                        nc.tensor.matmul(out=mm[:, lo:lo + cw],
                                         lhsT=vts[j][:],
                                         rhs=e0t[:, c0 + lo:c0 + lo + cw],
                                         start=True, stop=True)
                    eng = cast_cnt[0] % 2   # alternate scalar/vector
                    cast_cnt[0] += 1
                    cast(ob[:, c0:c0 + gw], mm[:, 0:gw], eng)
                    if gi % 2 == 1:
                        oc, ow = GROUPS[gi - 1][0], GROUPS[gi - 1][1] + gw
                        nc.sync.dma_start(
                            OUT[j * P:(j + 1) * P, oc:oc + ow],
                            ob[:, oc:oc + ow])

    nc.compile()
    return nc


def _prep_inputs(inputs):
    x = np.asarray(inputs["x"])
    E0 = np.asarray(inputs["E0"], dtype=np.float32)
    E1 = np.asarray(inputs["E1"], dtype=np.float32)
    E2 = np.asarray(inputs["E2"], dtype=np.float32)
    E3 = np.asarray(inputs["E3"], dtype=np.float32)
    E4 = np.asarray(inputs["E4"], dtype=np.float32)
    E5 = np.asarray(inputs["E5"], dtype=np.float32)
    E6 = np.asarray(inputs["E6"], dtype=np.float32)
    rule_C = np.asarray(inputs["rule_C"], dtype=np.float32)
    rule_S = np.asarray(inputs["rule_S"], dtype=np.float32)
    has_rules = np.asarray(inputs["has_rules"])

    h, r, t = (x[:, 0].astype(np.int64), x[:, 1].astype(np.int64),
               x[:, 3].astype(np.int64))
    tb = t // CYCLE
    H = RANK // 2

    L = E0[h]
    R = E1[r]
    RC = rule_C[r]
    CT = E4[t]
    TM = E2[t] + E5[tb]
    TE = E3[t] + E6[tb]
    hr = has_rules[r].astype(np.float32)
    hs = hr * rule_S[r]

    def hsp(a):
        return a[:, :H], a[:, H:]

    L0, L1 = hsp(L)
    R0, R1 = hsp(R)
    RC0, RC1 = hsp(RC)

    C0, C1 = hsp(CT)
    tblex = np.concatenate([
        R0, R0, R1, -R1,          # RELX4
        RC0, RC1, -RC1, RC0,      # RCP
        C0, C0, C1, C1,           # CTdup
        TM, TE,
        L0, L1, -L1, -L0,         # E0GX
        np.repeat(hr[:, None], RANK, axis=1),
        hs[:, None] * R,
    ], axis=1).astype(np.float16)   # [B, TW]
    assert tblex.shape[1] == TW

    e0t = np.ascontiguousarray(E0.T.astype(np.float16))   # [128, 40000]

    tbl_by_bh = []
    for bh in range(BS):
        rows = tblex[bh * ROWS:(bh + 1) * ROWS]
        tbl_by_bh.append(np.ascontiguousarray(
            rows.reshape(NT, P, TW).transpose(1, 0, 2)))
    e0t_by_es = [np.ascontiguousarray(e0t[:, es * NSLICE:(es + 1) * NSLICE])
                 for es in range(ES)]

    ident = np.eye(P, dtype=np.float16)
    in_maps = []
    for c in range(NCORES):
        in_maps.append({
            "TBL": tbl_by_bh[c // ES],
            "E0T": e0t_by_es[c % ES],
            "IDN": ident,
        })
    return in_maps


def kernel(**inputs):
    from concourse.bass_utils import run_bass_kernel_spmd

    if "nc" not in _CACHE:
        _CACHE["nc"] = _build()
    nc = _CACHE["nc"]

    in_maps = _prep_inputs(inputs)
    res = run_bass_kernel_spmd(nc, in_maps, core_ids=list(range(NCORES)),
                               trace=TRACE)
    _CACHE["last_result"] = res
    out = np.empty((B, NENT), np.float32)
    for c in range(NCORES):
        bh, es = c // ES, c % ES
        out[bh * ROWS:(bh + 1) * ROWS,
            es * NSLICE:(es + 1) * NSLICE] = res.results[c]["OUT"]
    return out


# revision 16
# speedup vs baseline: 1.3261x; 1.0712x over previous
"""Trainium2 Bass kernel for nn_CTRule (temporal KG scoring model).

Computes, for each of B=1024 queries (h, r, t):
  v = f(E0[h], E1[r], time tables, rule tables)   # [B, 128] elementwise algebra
  scores = v @ E0.T                               # [B, 40000]

Distribution over the 8 NeuronCores: 2-way batch x 4-way entity grid.
Core c handles batch rows [bh*512, bh*512+512) (bh = c//4) against entity
columns [es*10000, es*10000+10000) (es = c%4).  Per-core HBM traffic:
  out 10.24 MB + E0T slice 2.56 MB + tables ~1.2 MB  ->  ~39 us at the
358 GB/s per-core HBM limit, which (plus the ~8 us engine preamble) is the
kernel's floor.

Host prep: per-example table rows are pre-indexed on the host into one TBL
tensor ([128, 4 tiles, 1152] per core) laid out in the block patterns the
head algebra wants, so every complex/quaternion product is one wide fp16
multiply followed by a 128-wide "fold" add/sub:
  cmul(x, y)        = fold(+) of  [x0|x0|x1|x1] * [y0|y1|-y1|y0]
  complex_mul(x, y) = fold(+) of  [x0|x0|x1|-x1] * [y0|y1|y1|y0]
  mul4 tail         = fold(-/+) of Y * X1 and rev64(Y) * X1
has_rules / rule_S enter as per-partition f32 scalars (tensor_scalar).

Schedule (all engines near-saturated):
  * ALL input DMAs go on the sync HWDGE ring in dependency-latency order
    (tbl0, hrs, ident, e0t chunk0, tbl1, tbl2, tbl3, e0t bulk) — the two
    HWDGE rings share the 16 SDMA engines, so a second ring's bulk loads
    would delay the latency-critical table loads.  OUT chunks follow on
    the same FIFO; the ring never idles.
  * Heads: Vector computes tiles 0,1; GpSimd (slow but otherwise idle)
    computes tiles 2,3 concurrently.  v transposes on TensorE.
  * Scores: 512-col matmul chunks (PSUM-bank aligned!) into [P,1024] f32
    PSUM groups; groups drain via f32->fp16 casts alternating Scalar /
    Vector (GPSIMD cannot read PSUM); every 2 groups one [128,2048] OUT
    chunk is queued on the sync ring.
No cross-core communication; the host reassembles the 8 blocks.
"""

import numpy as np

P = 128
B = 1024
RANK = 128
NENT = 40000
NTIME = 365
CYCLE = 120
NCORES = 8
ES = 4                   # entity-axis splits
BS = 2                   # batch-axis splits
NSLICE = NENT // ES      # 10000 entity columns per core
ROWS = B // BS           # 512 rows per core
NT = ROWS // P           # 4 batch tiles per core
TW = 1536                # table width per tile (see column map below)
# matmul/cast groups: [P,1024] f32 = 2 PSUM banks; chunks must be 512-col
# bank-aligned (a 500-col chunk crossing a bank boundary corrupts results).
GROUPS = [(c, 1024) for c in range(0, 9216, 1024)] + [(9216, 784)]
GRP = 1024               # first E0T chunk width

# TBL column map (per tile):
C_RELX4 = 0      # [R0|R0|R1|-R1]           256
C_RCP = 256      # [RC0|RC1|-RC1|RC0]       256
C_CTD = 512      # [C0|C0|C1|C1] (CT dup)   256
C_TM = 768       # time = E2[t]+E5[tb]      128
C_TE = 896       # time_ent = E3[t]+E6[tb]  128
C_E0G = 1024     # [L0|L1|-L1|-L0]          256
C_HRW = 1280     # has_rules broadcast      128
C_HSR = 1408     # hr*rS*rel                128

TRACE = False            # set by test harness for profiling runs
_CACHE = {}


def _build():
    import concourse.bass as bass
    import concourse.mybir as mybir
    import concourse.tile as tile
    from concourse import bacc

    dt = mybir.dt
    mult = mybir.AluOpType.mult
    add = mybir.AluOpType.add
    sub = mybir.AluOpType.subtract

    nc = bacc.Bacc("TRN2", target_bir_lowering=False, debug=False,
                   num_devices=NCORES)

    TBL = nc.dram_tensor("TBL", [P, NT, TW], dt.float16, kind="ExternalInput").ap()
    E0T = nc.dram_tensor("E0T", [RANK, NSLICE], dt.float16, kind="ExternalInput").ap()
    IDN = nc.dram_tensor("IDN", [P, P], dt.float16, kind="ExternalInput").ap()
    OUT = nc.dram_tensor("OUT", [ROWS, NSLICE], dt.float16, kind="ExternalOutput").ap()

    def r4(ap):
        # view last dim as 4 blocks of 64
        return ap.rearrange("p t (s x) -> p t s x", s=4)

    def r2(ap):
        return ap.rearrange("p t (s x) -> p t s x", s=2)

    with tile.TileContext(nc) as tc:
        with (
            tc.tile_pool(name="const", bufs=1) as constp,
            tc.tile_pool(name="ew", bufs=1) as ew,
            tc.tile_pool(name="pst", bufs=1, space="PSUM") as pst,
            tc.tile_pool(name="psm", bufs=3, space="PSUM") as psm,
        ):
            tblt = [constp.tile([P, 1, TW], dt.float16, name=f"tbl{j}")
                    for j in range(NT)]
            e0t = constp.tile([RANK, NSLICE], dt.float16)
            ident = constp.tile([P, P], dt.float16)
            nc.sync.dma_start(tblt[0][:], TBL[:, 0:1, :])
            nc.sync.dma_start(ident[:], IDN[:])
            nc.sync.dma_start(e0t[:, 0:GRP], E0T[:, 0:GRP])
            nc.sync.dma_start(tblt[1][:], TBL[:, 1:2, :])
            nc.sync.dma_start(tblt[2][:], TBL[:, 2:3, :])
            nc.sync.dma_start(tblt[3][:], TBL[:, 3:4, :])
            nc.sync.dma_start(e0t[:, GRP:4096], E0T[:, GRP:4096])
            nc.sync.dma_start(e0t[:, 4096:7168], E0T[:, 4096:7168])
            nc.sync.dma_start(e0t[:, 7168:NSLICE], E0T[:, 7168:NSLICE])

            # ---- head: ~26 wide fp16 ops per tile (VectorE or GpSimd)
            def head(j, eng):
                t = tblt[j][:]               # [P, 1, TW]
                T = lambda a, b: t[:, :, a:b]
                nt = 1
                mk = lambda w, n: ew.tile([P, nt, w], dt.float16, name=f"{n}{j}")
                pa, pb, pc = mk(256, 'pa'), mk(256, 'pb'), mk(256, 'pc')
                fa, bt, bc = mk(128, 'fa'), mk(128, 'bt'), mk(128, 'bc')
                g = mk(128, 'g')
                w2, fc = mk(256, 'w2'), mk(128, 'fc')
                yy, x1 = mk(256, 'yy'), mk(256, 'x1')
                ma, mb = mk(256, 'ma'), mk(256, 'mb')
                fm, fn, vv = mk(128, 'fm'), mk(128, 'fn'), mk(128, 'vv')

                def TT(out, a, b, op):
                    eng.tensor_tensor(out=out, in0=a, in1=b, op=op)

                # rule branch: fa = cmul(CT, RC)
                TT(pa[:], T(C_CTD, C_CTD + 256), T(C_RCP, C_RCP + 256), mult)
                TT(fa[:], pa[:, :, 0:128], pa[:, :, 128:256], add)
                # no-rule branch: bt = lhs + cmul(rel, lhs)
                TT(pb[:], T(C_RELX4, C_RELX4 + 256), T(C_E0G, C_E0G + 256), mult)
                TT(bt[:], pb[:, :, 0:128], pb[:, :, 128:256], add)
                TT(bt[:], bt[:], T(C_E0G, C_E0G + 128), add)
                # bc = bt + CT (CT = blocks {0,2} of CTdup)
                TT(r2(bc[:]), r2(bt[:]),
                   r4(T(C_CTD, C_CTD + 256))[:, :, 0::2, :], add)
                # w = hr*(fa - bt) - hr*rS*rel + bt + CT
                TT(g[:], fa[:], bt[:], sub)
                TT(g[:], g[:], T(C_HRW, C_HRW + 128), mult)
                TT(g[:], g[:], T(C_HSR, C_HSR + 128), sub)
                TT(w2[:, :, 0:128], g[:], bc[:], add)
                eng.tensor_copy(out=r2(w2[:, :, 128:256]),
                                in_=r2(w2[:, :, 0:128])[:, :, ::-1, :])
                # rel_ = rel + complex_mul(rel, w) -> Y blocks {0,2}
                TT(pc[:], T(C_RELX4, C_RELX4 + 256), w2[:], mult)
                TT(fc[:], pc[:, :, 0:128], pc[:, :, 128:256], add)
                TT(r4(yy[:])[:, :, 0::2, :], r2(fc[:]),
                   r4(T(C_RELX4, C_RELX4 + 256))[:, :, 0::2, :], add)
                # Y blocks {1,3} = TM halves
                eng.tensor_copy(out=r4(yy[:])[:, :, 1::2, :],
                                in_=r2(T(C_TM, C_TM + 128)))
                # X1 = [L0+T0 | L0-T0 | L1-T1 | L1+T1]
                TT(r4(x1[:])[:, :, 0::3, :], r2(T(C_E0G, C_E0G + 128)),
                   r2(T(C_TE, C_TE + 128)), add)
                TT(r4(x1[:])[:, :, 1:3, :], r2(T(C_E0G, C_E0G + 128)),
                   r2(T(C_TE, C_TE + 128)), sub)
                # v
                TT(ma[:], yy[:], x1[:], mult)
                TT(mb[:], r4(yy[:])[:, :, ::-1, :], x1[:], mult)
                TT(fm[:], ma[:, :, 0:128], ma[:, :, 128:256], sub)
                TT(vv[:, :, 0:64], fm[:, :, 0:64], fm[:, :, 64:128], add)
                TT(fn[:], mb[:, :, 0:128], mb[:, :, 128:256], add)
                TT(vv[:, :, 64:128], fn[:, :, 0:64], fn[:, :, 64:128], add)
                return vv

            # ALL heads on Vector: any GpSimd activity drops DVE out of its
            # 2-port perf mode (~2x slower vector ops), so GpSimd stays idle.
            # Head emission interleaves with the cast loop so Vector's cast
            # share flows early enough to keep the OUT ring fed.
            vts = {}

            def finish_vt(vsrc, j):
                vt_ps = pst.tile([P, P], dt.float16, space="PSUM", tag="vtps")
                nc.tensor.transpose(out=vt_ps[:], in_=vsrc[:, 0, :],
                                    identity=ident[:])
                vt = constp.tile([P, P], dt.float16, name=f"vt{j}")
                nc.vector.tensor_copy(out=vt[:], in_=vt_ps[:])
                vts[j] = vt

            v0 = head(0, nc.vector)
            finish_vt(v0, 0)
            v1 = head(1, nc.vector)

            osb = [constp.tile([P, NSLICE], dt.float16, name=f"osb{i}")
                   for i in range(NT)]
            # cast engine per (tile, group): Scalar carries most early casts
            # (Vector still computing heads); Vector joins where marked.
            VCAST = {(0, 1), (0, 3), (1, 1), (1, 3),
                     (2, 1), (2, 3), (2, 5), (2, 7), (2, 9),
                     (3, 1), (3, 3), (3, 5), (3, 7), (3, 9)}

            vheads = {1: lambda: None, 2: lambda: head(2, nc.vector),
                      3: lambda: head(3, nc.vector)}
            vsrcs = {0: v0, 1: v1}
            for j in range(NT):
                if j >= 1:
                    # transpose just before this tile's matmuls: the in-order
                    # tensor engine must not park on a not-yet-ready head
                    finish_vt(vsrcs[j], j)
                ob = osb[j]
                for gi, (c0, gw) in enumerate(GROUPS):
                    mm = psm.tile([P, 1024], dt.float32, space="PSUM", tag="mm")
                    for lo in range(0, gw, 512):
                        cw = min(512, gw - lo)
                        nc.tensor.matmul(out=mm[:, lo:lo + cw],
                                         lhsT=vts[j][:],
                                         rhs=e0t[:, c0 + lo:c0 + lo + cw],
                                         start=True, stop=True)
                    if (j, gi) in VCAST:
                        nc.vector.tensor_copy(out=ob[:, c0:c0 + gw],
                                              in_=mm[:, 0:gw])
                    else:
                        nc.scalar.copy(out=ob[:, c0:c0 + gw], in_=mm[:, 0:gw])
                    if gi % 2 == 1:
                        oc, ow = GROUPS[gi - 1][0], GROUPS[gi - 1][1] + gw
                        nc.sync.dma_start(
                            OUT[j * P:(j + 1) * P, oc:oc + ow],
                            ob[:, oc:oc + ow])
                # next tile's head emitted AFTER this tile's vector casts so
                # the vector queue interleaves heads with early cast work
                if j + 1 <= 3 and j + 1 >= 2:
                    vsrcs[j + 1] = vheads[j + 1]()

    nc.compile()
    return nc


def _prep_inputs(inputs):
    x = np.asarray(inputs["x"])
    E0 = np.asarray(inputs["E0"], dtype=np.float32)
    E1 = np.asarray(inputs["E1"], dtype=np.float32)
    E2 = np.asarray(inputs["E2"], dtype=np.float32)
    E3 = np.asarray(inputs["E3"], dtype=np.float32)
    E4 = np.asarray(inputs["E4"], dtype=np.float32)
    E5 = np.asarray(inputs["E5"], dtype=np.float32)
    E6 = np.asarray(inputs["E6"], dtype=np.float32)
    rule_C = np.asarray(inputs["rule_C"], dtype=np.float32)
    rule_S = np.asarray(inputs["rule_S"], dtype=np.float32)
    has_rules = np.asarray(inputs["has_rules"])

    h, r, t = (x[:, 0].astype(np.int64), x[:, 1].astype(np.int64),
               x[:, 3].astype(np.int64))
    tb = t // CYCLE
    H = RANK // 2

    L = E0[h]
    R = E1[r]
    RC = rule_C[r]
    CT = E4[t]
    TM = E2[t] + E5[tb]
    TE = E3[t] + E6[tb]
    hr = has_rules[r].astype(np.float32)
    hs = hr * rule_S[r]

    def hsp(a):
        return a[:, :H], a[:, H:]

    L0, L1 = hsp(L)
    R0, R1 = hsp(R)
    RC0, RC1 = hsp(RC)

    C0, C1 = hsp(CT)
    tblex = np.concatenate([
        R0, R0, R1, -R1,          # RELX4
        RC0, RC1, -RC1, RC0,      # RCP
        C0, C0, C1, C1,           # CTdup
        TM, TE,
        L0, L1, -L1, -L0,         # E0GX
        np.repeat(hr[:, None], RANK, axis=1),
        hs[:, None] * R,
    ], axis=1).astype(np.float16)   # [B, TW]
    assert tblex.shape[1] == TW

    e0t = np.ascontiguousarray(E0.T.astype(np.float16))   # [128, 40000]

    tbl_by_bh = []
    for bh in range(BS):
        rows = tblex[bh * ROWS:(bh + 1) * ROWS]
        tbl_by_bh.append(np.ascontiguousarray(
            rows.reshape(NT, P, TW).transpose(1, 0, 2)))
    e0t_by_es = [np.ascontiguousarray(e0t[:, es * NSLICE:(es + 1) * NSLICE])
                 for es in range(ES)]

    ident = np.eye(P, dtype=np.float16)
    in_maps = []
    for c in range(NCORES):
        in_maps.append({
            "TBL": tbl_by_bh[c // ES],
            "E0T": e0t_by_es[c % ES],
            "IDN": ident,
        })
    return in_maps


def kernel(**inputs):
    from concourse.bass_utils import run_bass_kernel_spmd

    if "nc" not in _CACHE:
        _CACHE["nc"] = _build()
    nc = _CACHE["nc"]

    in_maps = _prep_inputs(inputs)
    res = run_bass_kernel_spmd(nc, in_maps, core_ids=list(range(NCORES)),
                               trace=TRACE)
    _CACHE["last_result"] = res
    out = np.empty((B, NENT), np.float32)
    for c in range(NCORES):
        bh, es = c // ES, c % ES
        out[bh * ROWS:(bh + 1) * ROWS,
            es * NSLICE:(es + 1) * NSLICE] = res.results[c]["OUT"]
    return out


# revision 17
# speedup vs baseline: 1.4404x; 1.0862x over previous
"""Trainium2 Bass kernel for nn_CTRule (temporal KG scoring model).

Computes, for each of B=1024 queries (h, r, t):
  v = f(E0[h], E1[r], time tables, rule tables)   # [B, 128] elementwise algebra
  scores = v @ E0.T                               # [B, 40000]

Distribution over the 8 NeuronCores: 2-way batch x 4-way entity grid.
Core c handles batch rows [bh*512, bh*512+512) (bh = c//4) against entity
columns [es*10000, es*10000+10000) (es = c%4).  Per-core HBM traffic:
  out 10.24 MB + E0T slice 2.56 MB + tables ~1.2 MB  ->  ~39 us at the
358 GB/s per-core HBM limit, which (plus the ~8 us engine preamble) is the
kernel's floor.

Host prep: per-example table rows are pre-indexed on the host into one TBL
tensor ([128, 4 tiles, 1152] per core) laid out in the block patterns the
head algebra wants, so every complex/quaternion product is one wide fp16
multiply followed by a 128-wide "fold" add/sub:
  cmul(x, y)        = fold(+) of  [x0|x0|x1|x1] * [y0|y1|-y1|y0]
  complex_mul(x, y) = fold(+) of  [x0|x0|x1|-x1] * [y0|y1|y1|y0]
  mul4 tail         = fold(-/+) of Y * X1 and rev64(Y) * X1
has_rules / rule_S enter as per-partition f32 scalars (tensor_scalar).

Schedule (all engines near-saturated):
  * ALL input DMAs go on the sync HWDGE ring in dependency-latency order
    (tbl0, hrs, ident, e0t chunk0, tbl1, tbl2, tbl3, e0t bulk) — the two
    HWDGE rings share the 16 SDMA engines, so a second ring's bulk loads
    would delay the latency-critical table loads.  OUT chunks follow on
    the same FIFO; the ring never idles.
  * Heads: Vector computes tiles 0,1; GpSimd (slow but otherwise idle)
    computes tiles 2,3 concurrently.  v transposes on TensorE.
  * Scores: 512-col matmul chunks (PSUM-bank aligned!) into [P,1024] f32
    PSUM groups; groups drain via f32->fp16 casts alternating Scalar /
    Vector (GPSIMD cannot read PSUM); every 2 groups one [128,2048] OUT
    chunk is queued on the sync ring.
No cross-core communication; the host reassembles the 8 blocks.
"""

import numpy as np

P = 128
B = 1024
RANK = 128
NENT = 40000
NTIME = 365
CYCLE = 120
NCORES = 8
ES = 4                   # entity-axis splits
BS = 2                   # batch-axis splits
NSLICE = NENT // ES      # 10000 entity columns per core
ROWS = B // BS           # 512 rows per core
NT = ROWS // P           # 4 batch tiles per core
TW = 1536                # table width per tile (see column map below)
# matmul/cast groups: [P,1024] f32 = 2 PSUM banks; chunks must be 512-col
# bank-aligned (a 500-col chunk crossing a bank boundary corrupts results).
GROUPS = [(c, 1024) for c in range(0, 9216, 1024)] + [(9216, 784)]
GRP = 1024               # first E0T chunk width

# TBL column map (per tile):
C_RELX4 = 0      # [R0|R0|R1|-R1]           256
C_RCP = 256      # [RC0|RC1|-RC1|RC0]       256
C_CTD = 512      # [C0|C0|C1|C1] (CT dup)   256
C_TM = 768       # time = E2[t]+E5[tb]      128
C_TE = 896       # time_ent = E3[t]+E6[tb]  128
C_E0G = 1024     # [L0|L1|-L1|-L0]          256
C_HRW = 1280     # has_rules broadcast      128
C_HSR = 1408     # hr*rS*rel                128

TRACE = False            # set by test harness for profiling runs
_CACHE = {}


def _build():
    import concourse.bass as bass
    import concourse.mybir as mybir
    import concourse.tile as tile
    from concourse import bacc

    dt = mybir.dt
    mult = mybir.AluOpType.mult
    add = mybir.AluOpType.add
    sub = mybir.AluOpType.subtract

    nc = bacc.Bacc("TRN2", target_bir_lowering=False, debug=False,
                   num_devices=NCORES)

    TBL = nc.dram_tensor("TBL", [P, NT, TW], dt.float16, kind="ExternalInput").ap()
    E0T = nc.dram_tensor("E0T", [RANK, NSLICE], dt.float16, kind="ExternalInput").ap()
    IDN = nc.dram_tensor("IDN", [P, P], dt.float16, kind="ExternalInput").ap()
    OUT = nc.dram_tensor("OUT", [ROWS, NSLICE], dt.float16, kind="ExternalOutput").ap()

    def r4(ap):
        # view last dim as 4 blocks of 64
        return ap.rearrange("p t (s x) -> p t s x", s=4)

    def r2(ap):
        return ap.rearrange("p t (s x) -> p t s x", s=2)

    with tile.TileContext(nc) as tc:
        with (
            tc.tile_pool(name="const", bufs=1) as constp,
            tc.tile_pool(name="ew", bufs=1) as ew,
            tc.tile_pool(name="pst", bufs=1, space="PSUM") as pst,
            tc.tile_pool(name="psm", bufs=3, space="PSUM") as psm,
        ):
            tbl0 = constp.tile([P, 1, TW], dt.float16, name="tbl0")
            tbl123 = constp.tile([P, 3, TW], dt.float16, name="tbl123")
            e0t = constp.tile([RANK, NSLICE], dt.float16)
            ident = constp.tile([P, P], dt.float16)
            nc.sync.dma_start(tbl0[:], TBL[:, 0:1, :])
            nc.sync.dma_start(ident[:], IDN[:])
            nc.sync.dma_start(e0t[:, 0:GRP], E0T[:, 0:GRP])
            nc.sync.dma_start(tbl123[:], TBL[:, 1:4, :])
            nc.sync.dma_start(e0t[:, GRP:4096], E0T[:, GRP:4096])
            nc.sync.dma_start(e0t[:, 4096:7168], E0T[:, 4096:7168])
            nc.sync.dma_start(e0t[:, 7168:NSLICE], E0T[:, 7168:NSLICE])

            # ---- head: ~26 wide fp16 ops per tile (VectorE or GpSimd)
            def head(tag, tsrc, nt, eng):
                t = tsrc[:]                  # [P, nt, TW]
                T = lambda a, b: t[:, :, a:b]
                mk = lambda w, n: ew.tile([P, nt, w], dt.float16, name=f"{n}{tag}")
                pa, pb, pc = mk(256, 'pa'), mk(256, 'pb'), mk(256, 'pc')
                fa, bt, bc = mk(128, 'fa'), mk(128, 'bt'), mk(128, 'bc')
                g = mk(128, 'g')
                w2, fc = mk(256, 'w2'), mk(128, 'fc')
                yy, x1 = mk(256, 'yy'), mk(256, 'x1')
                ma, mb = mk(256, 'ma'), mk(256, 'mb')
                fm, fn, vv = mk(128, 'fm'), mk(128, 'fn'), mk(128, 'vv')

                def TT(out, a, b, op):
                    eng.tensor_tensor(out=out, in0=a, in1=b, op=op)

                # rule branch: fa = cmul(CT, RC)
                TT(pa[:], T(C_CTD, C_CTD + 256), T(C_RCP, C_RCP + 256), mult)
                TT(fa[:], pa[:, :, 0:128], pa[:, :, 128:256], add)
                # no-rule branch: bt = lhs + cmul(rel, lhs)
                TT(pb[:], T(C_RELX4, C_RELX4 + 256), T(C_E0G, C_E0G + 256), mult)
                TT(bt[:], pb[:, :, 0:128], pb[:, :, 128:256], add)
                TT(bt[:], bt[:], T(C_E0G, C_E0G + 128), add)
                # bc = bt + CT (CT = blocks {0,2} of CTdup)
                TT(r2(bc[:]), r2(bt[:]),
                   r4(T(C_CTD, C_CTD + 256))[:, :, 0::2, :], add)
                # w = hr*(fa - bt) - hr*rS*rel + bt + CT
                TT(g[:], fa[:], bt[:], sub)
                TT(g[:], g[:], T(C_HRW, C_HRW + 128), mult)
                TT(g[:], g[:], T(C_HSR, C_HSR + 128), sub)
                TT(w2[:, :, 0:128], g[:], bc[:], add)
                eng.tensor_copy(out=r2(w2[:, :, 128:256]),
                                in_=r2(w2[:, :, 0:128])[:, :, ::-1, :])
                # rel_ = rel + complex_mul(rel, w) -> Y blocks {0,2}
                TT(pc[:], T(C_RELX4, C_RELX4 + 256), w2[:], mult)
                TT(fc[:], pc[:, :, 0:128], pc[:, :, 128:256], add)
                TT(r4(yy[:])[:, :, 0::2, :], r2(fc[:]),
                   r4(T(C_RELX4, C_RELX4 + 256))[:, :, 0::2, :], add)
                # Y blocks {1,3} = TM halves
                eng.tensor_copy(out=r4(yy[:])[:, :, 1::2, :],
                                in_=r2(T(C_TM, C_TM + 128)))
                # X1 = [L0+T0 | L0-T0 | L1-T1 | L1+T1]
                TT(r4(x1[:])[:, :, 0::3, :], r2(T(C_E0G, C_E0G + 128)),
                   r2(T(C_TE, C_TE + 128)), add)
                TT(r4(x1[:])[:, :, 1:3, :], r2(T(C_E0G, C_E0G + 128)),
                   r2(T(C_TE, C_TE + 128)), sub)
                # v
                TT(ma[:], yy[:], x1[:], mult)
                TT(mb[:], r4(yy[:])[:, :, ::-1, :], x1[:], mult)
                TT(fm[:], ma[:, :, 0:128], ma[:, :, 128:256], sub)
                TT(vv[:, :, 0:64], fm[:, :, 0:64], fm[:, :, 64:128], add)
                TT(fn[:], mb[:, :, 0:128], mb[:, :, 128:256], add)
                TT(vv[:, :, 64:128], fn[:, :, 0:64], fn[:, :, 64:128], add)
                return vv

            # ALL heads on Vector: any GpSimd activity drops DVE out of its
            # 2-port perf mode (~2x slower vector ops), so GpSimd stays idle.
            # Tile 0 computes alone (gates the whole pipeline); tiles 1-3
            # batch as one [P,3,*] group (one op sweep, ~half the time).
            vts = {}

            def emit_transpose(vsrc, k, j):
                vt_ps = pst.tile([P, P], dt.float16, space="PSUM", tag="vtps")
                nc.tensor.transpose(out=vt_ps[:], in_=vsrc[:, k, :],
                                    identity=ident[:])
                vt = constp.tile([P, P], dt.float16, name=f"vt{j}")
                nc.vector.tensor_copy(out=vt[:], in_=vt_ps[:])
                vts[j] = vt

            v0 = head(0, tbl0, 1, nc.vector)
            emit_transpose(v0, 0, 0)
            v123 = head(1, tbl123, 3, nc.vector)

            osb = [constp.tile([P, NSLICE], dt.float16, name=f"osb{i}")
                   for i in range(NT)]
            # cast engine per (tile, group): Scalar carries the early casts
            # (Vector still computing heads); Vector joins from (0,9) on.
            VCAST = {(0, 9)} | {(j, gi) for j in (1, 2, 3)
                                for gi in (1, 3, 5, 7, 9)}

            for j in range(NT):
                ob = osb[j]
                for gi, (c0, gw) in enumerate(GROUPS):
                    mm = psm.tile([P, 1024], dt.float32, space="PSUM", tag="mm")
                    for lo in range(0, gw, 512):
                        cw = min(512, gw - lo)
                        nc.tensor.matmul(out=mm[:, lo:lo + cw],
                                         lhsT=vts[j][:],
                                         rhs=e0t[:, c0 + lo:c0 + lo + cw],
                                         start=True, stop=True)
                    if (j, gi) in VCAST:
                        nc.vector.tensor_copy(out=ob[:, c0:c0 + gw],
                                              in_=mm[:, 0:gw])
                    else:
                        nc.scalar.copy(out=ob[:, c0:c0 + gw], in_=mm[:, 0:gw])
                    if gi % 2 == 1:
                        oc, ow = GROUPS[gi - 1][0], GROUPS[gi - 1][1] + gw
                        nc.sync.dma_start(
                            OUT[j * P:(j + 1) * P, oc:oc + ow],
                            ob[:, oc:oc + ow])
                    if j == 0 and gi == 7:
                        # transposes for tiles 1-3 slot in here: late enough
                        # that the in-order tensor engine barely parks on
                        # v123, early enough that tile-1 matmuls are unblocked
                        for k in range(3):
                            emit_transpose(v123, k, k + 1)

    nc.compile()
    return nc


def _prep_inputs(inputs):
    x = np.asarray(inputs["x"])
    E0 = np.asarray(inputs["E0"], dtype=np.float32)
    E1 = np.asarray(inputs["E1"], dtype=np.float32)
    E2 = np.asarray(inputs["E2"], dtype=np.float32)
    E3 = np.asarray(inputs["E3"], dtype=np.float32)
    E4 = np.asarray(inputs["E4"], dtype=np.float32)
    E5 = np.asarray(inputs["E5"], dtype=np.float32)
    E6 = np.asarray(inputs["E6"], dtype=np.float32)
    rule_C = np.asarray(inputs["rule_C"], dtype=np.float32)
    rule_S = np.asarray(inputs["rule_S"], dtype=np.float32)
    has_rules = np.asarray(inputs["has_rules"])

    h, r, t = (x[:, 0].astype(np.int64), x[:, 1].astype(np.int64),
               x[:, 3].astype(np.int64))
    tb = t // CYCLE
    H = RANK // 2

    L = E0[h]
    R = E1[r]
    RC = rule_C[r]
    CT = E4[t]
    TM = E2[t] + E5[tb]
    TE = E3[t] + E6[tb]
    hr = has_rules[r].astype(np.float32)
    hs = hr * rule_S[r]

    def hsp(a):
        return a[:, :H], a[:, H:]

    L0, L1 = hsp(L)
    R0, R1 = hsp(R)
    RC0, RC1 = hsp(RC)

    C0, C1 = hsp(CT)
    tblex = np.concatenate([
        R0, R0, R1, -R1,          # RELX4
        RC0, RC1, -RC1, RC0,      # RCP
        C0, C0, C1, C1,           # CTdup
        TM, TE,
        L0, L1, -L1, -L0,         # E0GX
        np.repeat(hr[:, None], RANK, axis=1),
        hs[:, None] * R,
    ], axis=1).astype(np.float16)   # [B, TW]
    assert tblex.shape[1] == TW

    e0t = np.ascontiguousarray(E0.T.astype(np.float16))   # [128, 40000]

    tbl_by_bh = []
    for bh in range(BS):
        rows = tblex[bh * ROWS:(bh + 1) * ROWS]
        tbl_by_bh.append(np.ascontiguousarray(
            rows.reshape(NT, P, TW).transpose(1, 0, 2)))
    e0t_by_es = [np.ascontiguousarray(e0t[:, es * NSLICE:(es + 1) * NSLICE])
                 for es in range(ES)]

    ident = np.eye(P, dtype=np.float16)
    in_maps = []
    for c in range(NCORES):
        in_maps.append({
            "TBL": tbl_by_bh[c // ES],
            "E0T": e0t_by_es[c % ES],
            "IDN": ident,
        })
    return in_maps


def kernel(**inputs):
    from concourse.bass_utils import run_bass_kernel_spmd

    if "nc" not in _CACHE:
        _CACHE["nc"] = _build()
    nc = _CACHE["nc"]

    in_maps = _prep_inputs(inputs)
    res = run_bass_kernel_spmd(nc, in_maps, core_ids=list(range(NCORES)),
                               trace=TRACE)
    _CACHE["last_result"] = res
    out = np.empty((B, NENT), np.float32)
    for c in range(NCORES):
        bh, es = c // ES, c % ES
        out[bh * ROWS:(bh + 1) * ROWS,
            es * NSLICE:(es + 1) * NSLICE] = res.results[c]["OUT"]
    return out


# revision 19
# speedup vs baseline: 1.5133x; 1.0506x over previous
"""Trainium2 Bass kernel for nn_CTRule (temporal KG scoring model).

Computes, for each of B=1024 queries (h, r, t):
  v = f(E0[h], E1[r], time tables, rule tables)   # [B, 128] elementwise algebra
  scores = v @ E0.T                               # [B, 40000]

Distribution over the 8 NeuronCores: 2-way batch x 4-way entity grid.
Core c handles batch rows [bh*512, bh*512+512) (bh = c//4) against entity
columns [es*10000, es*10000+10000) (es = c%4).  Per-core HBM traffic:
  out 10.24 MB + E0T slice 2.56 MB + tables ~1.2 MB  ->  ~39 us at the
358 GB/s per-core HBM limit, which (plus the ~8 us engine preamble) is the
kernel's floor.

Host prep: per-example table rows are pre-indexed on the host into one TBL
tensor ([128, 4 tiles, 1152] per core) laid out in the block patterns the
head algebra wants, so every complex/quaternion product is one wide fp16
multiply followed by a 128-wide "fold" add/sub:
  cmul(x, y)        = fold(+) of  [x0|x0|x1|x1] * [y0|y1|-y1|y0]
  complex_mul(x, y) = fold(+) of  [x0|x0|x1|-x1] * [y0|y1|y1|y0]
  mul4 tail         = fold(-/+) of Y * X1 and rev64(Y) * X1
has_rules / rule_S enter as per-partition f32 scalars (tensor_scalar).

Schedule (all engines near-saturated):
  * ALL input DMAs go on the sync HWDGE ring in dependency-latency order
    (tbl0, hrs, ident, e0t chunk0, tbl1, tbl2, tbl3, e0t bulk) — the two
    HWDGE rings share the 16 SDMA engines, so a second ring's bulk loads
    would delay the latency-critical table loads.  OUT chunks follow on
    the same FIFO; the ring never idles.
  * Heads: Vector computes tiles 0,1; GpSimd (slow but otherwise idle)
    computes tiles 2,3 concurrently.  v transposes on TensorE.
  * Scores: 512-col matmul chunks (PSUM-bank aligned!) into [P,1024] f32
    PSUM groups; groups drain via f32->fp16 casts alternating Scalar /
    Vector (GPSIMD cannot read PSUM); every 2 groups one [128,2048] OUT
    chunk is queued on the sync ring.
No cross-core communication; the host reassembles the 8 blocks.
"""

import numpy as np

P = 128
B = 1024
RANK = 128
NENT = 40000
NTIME = 365
CYCLE = 120
NCORES = 8
ES = 4                   # entity-axis splits
BS = 2                   # batch-axis splits
NSLICE = NENT // ES      # 10000 entity columns per core
ROWS = B // BS           # 512 rows per core
NT = ROWS // P           # 4 batch tiles per core
TW = 1536                # table width per tile (see column map below)
# matmul/cast groups: [P,1024] f32 = 2 PSUM banks; chunks must be 512-col
# bank-aligned (a 500-col chunk crossing a bank boundary corrupts results).
GROUPS = [(c, 1024) for c in range(0, 9216, 1024)] + [(9216, 784)]
GRP = 1024               # first E0T chunk width

# TBL column map (per tile):
C_RELX4 = 0      # [R0|R0|R1|-R1]           256
C_RCP = 256      # [RC0|RC1|-RC1|RC0]       256
C_CTD = 512      # [C0|C0|C1|C1] (CT dup)   256
C_TM = 768       # time = E2[t]+E5[tb]      128
C_TE = 896       # time_ent = E3[t]+E6[tb]  128
C_E0G = 1024     # [L0|L1|-L1|-L0]          256
C_HRW = 1280     # has_rules broadcast      128
C_HSR = 1408     # hr*rS*rel                128

TRACE = False            # set by test harness for profiling runs
_CACHE = {}


def _build():
    import concourse.bass as bass
    import concourse.mybir as mybir
    import concourse.tile as tile
    from concourse import bacc

    dt = mybir.dt
    mult = mybir.AluOpType.mult
    add = mybir.AluOpType.add
    sub = mybir.AluOpType.subtract

    nc = bacc.Bacc("TRN2", target_bir_lowering=False, debug=False,
                   num_devices=NCORES)

    TBL = nc.dram_tensor("TBL", [P, NT, TW], dt.float16, kind="ExternalInput").ap()
    E0T = nc.dram_tensor("E0T", [RANK, NSLICE], dt.float16, kind="ExternalInput").ap()
    IDN = nc.dram_tensor("IDN", [P, P], dt.float16, kind="ExternalInput").ap()
    OUT = nc.dram_tensor("OUT", [ROWS, NSLICE], dt.float16, kind="ExternalOutput").ap()

    def r4(ap):
        # view last dim as 4 blocks of 64
        return ap.rearrange("p t (s x) -> p t s x", s=4)

    def r2(ap):
        return ap.rearrange("p t (s x) -> p t s x", s=2)

    with tile.TileContext(nc) as tc:
        with (
            tc.tile_pool(name="const", bufs=1) as constp,
            tc.tile_pool(name="ew", bufs=1) as ew,
            tc.tile_pool(name="pst", bufs=1, space="PSUM") as pst,
            tc.tile_pool(name="psm", bufs=3, space="PSUM") as psm,
        ):
            tbl0 = constp.tile([P, 1, TW], dt.float16, name="tbl0")
            tbl123 = constp.tile([P, 3, TW], dt.float16, name="tbl123")
            e0t = constp.tile([RANK, NSLICE], dt.float16)
            ident = constp.tile([P, P], dt.float16)
            nc.sync.dma_start(tbl0[:], TBL[:, 0:1, :])
            nc.sync.dma_start(ident[:], IDN[:])
            nc.sync.dma_start(e0t[:, 0:GRP], E0T[:, 0:GRP])
            nc.sync.dma_start(tbl123[:], TBL[:, 1:4, :])
            nc.sync.dma_start(e0t[:, GRP:4096], E0T[:, GRP:4096])
            nc.sync.dma_start(e0t[:, 4096:7168], E0T[:, 4096:7168])
            nc.sync.dma_start(e0t[:, 7168:NSLICE], E0T[:, 7168:NSLICE])

            # ---- head: ~26 wide fp16 ops per tile (VectorE or GpSimd)
            def head(tag, t, nt, eng):
                # t: AP of shape [P, nt, TW]
                T = lambda a, b: t[:, :, a:b]
                mk = lambda w, n: ew.tile([P, nt, w], dt.float16, name=f"{n}{tag}")
                pa, pb, pc = mk(256, 'pa'), mk(256, 'pb'), mk(256, 'pc')
                fa, bt, bc = mk(128, 'fa'), mk(128, 'bt'), mk(128, 'bc')
                g = mk(128, 'g')
                w2, fc = mk(256, 'w2'), mk(128, 'fc')
                yy, x1 = mk(256, 'yy'), mk(256, 'x1')
                ma, mb = mk(256, 'ma'), mk(256, 'mb')
                fm, fn, vv = mk(128, 'fm'), mk(128, 'fn'), mk(128, 'vv')

                def TT(out, a, b, op):
                    eng.tensor_tensor(out=out, in0=a, in1=b, op=op)

                # rule branch: fa = cmul(CT, RC)
                TT(pa[:], T(C_CTD, C_CTD + 256), T(C_RCP, C_RCP + 256), mult)
                TT(fa[:], pa[:, :, 0:128], pa[:, :, 128:256], add)
                # no-rule branch: bt = lhs + cmul(rel, lhs)
                TT(pb[:], T(C_RELX4, C_RELX4 + 256), T(C_E0G, C_E0G + 256), mult)
                TT(bt[:], pb[:, :, 0:128], pb[:, :, 128:256], add)
                TT(bt[:], bt[:], T(C_E0G, C_E0G + 128), add)
                # bc = bt + CT (CT = blocks {0,2} of CTdup)
                TT(r2(bc[:]), r2(bt[:]),
                   r4(T(C_CTD, C_CTD + 256))[:, :, 0::2, :], add)
                # w = hr*(fa - bt) - hr*rS*rel + bt + CT
                TT(g[:], fa[:], bt[:], sub)
                TT(g[:], g[:], T(C_HRW, C_HRW + 128), mult)
                TT(g[:], g[:], T(C_HSR, C_HSR + 128), sub)
                TT(w2[:, :, 0:128], g[:], bc[:], add)
                eng.tensor_copy(out=r2(w2[:, :, 128:256]),
                                in_=r2(w2[:, :, 0:128])[:, :, ::-1, :])
                # rel_ = rel + complex_mul(rel, w) -> Y blocks {0,2}
                TT(pc[:], T(C_RELX4, C_RELX4 + 256), w2[:], mult)
                TT(fc[:], pc[:, :, 0:128], pc[:, :, 128:256], add)
                TT(r4(yy[:])[:, :, 0::2, :], r2(fc[:]),
                   r4(T(C_RELX4, C_RELX4 + 256))[:, :, 0::2, :], add)
                # Y blocks {1,3} = TM halves
                eng.tensor_copy(out=r4(yy[:])[:, :, 1::2, :],
                                in_=r2(T(C_TM, C_TM + 128)))
                # X1 = [L0+T0 | L0-T0 | L1-T1 | L1+T1]
                TT(r4(x1[:])[:, :, 0::3, :], r2(T(C_E0G, C_E0G + 128)),
                   r2(T(C_TE, C_TE + 128)), add)
                TT(r4(x1[:])[:, :, 1:3, :], r2(T(C_E0G, C_E0G + 128)),
                   r2(T(C_TE, C_TE + 128)), sub)
                # v
                TT(ma[:], yy[:], x1[:], mult)
                TT(mb[:], r4(yy[:])[:, :, ::-1, :], x1[:], mult)
                TT(fm[:], ma[:, :, 0:128], ma[:, :, 128:256], sub)
                TT(vv[:, :, 0:64], fm[:, :, 0:64], fm[:, :, 64:128], add)
                TT(fn[:], mb[:, :, 0:128], mb[:, :, 128:256], add)
                TT(vv[:, :, 64:128], fn[:, :, 0:64], fn[:, :, 64:128], add)
                return vv

            # ALL heads on Vector: any GpSimd activity drops DVE out of its
            # 2-port perf mode (~2x slower vector ops), so GpSimd stays idle.
            # Tile 0 computes alone (gates the whole pipeline); tiles 1-3
            # batch as one [P,3,*] group (one op sweep, ~half the time).
            vts = {}

            def emit_transpose(vsrc, k, j):
                vt_ps = pst.tile([P, P], dt.float16, space="PSUM", tag="vtps")
                nc.tensor.transpose(out=vt_ps[:], in_=vsrc[:, k, :],
                                    identity=ident[:])
                vt = constp.tile([P, P], dt.float16, name=f"vt{j}")
                nc.vector.tensor_copy(out=vt[:], in_=vt_ps[:])
                vts[j] = vt

            v0 = head(0, tbl0[:], 1, nc.vector)
            emit_transpose(v0, 0, 0)
            v12 = head(1, tbl123[:, 0:2, :], 2, nc.vector)
            v3 = None

            osb = [constp.tile([P, NSLICE], dt.float16, name=f"osb{i}")
                   for i in range(NT)]
            # cast engine per (tile, group): Scalar carries the early casts
            # (Vector still computing heads); Vector joins from (0,9) on.
            VCAST = {(0, 7), (0, 9)} | {(j, gi) for j in (1, 2, 3)
                                        for gi in (1, 3, 5, 7, 9)}

            for j in range(NT):
                ob = osb[j]
                for gi, (c0, gw) in enumerate(GROUPS):
                    mm = psm.tile([P, 1024], dt.float32, space="PSUM", tag="mm")
                    for lo in range(0, gw, 512):
                        cw = min(512, gw - lo)
                        nc.tensor.matmul(out=mm[:, lo:lo + cw],
                                         lhsT=vts[j][:],
                                         rhs=e0t[:, c0 + lo:c0 + lo + cw],
                                         start=True, stop=True)
                    if (j, gi) in VCAST:
                        nc.vector.tensor_copy(out=ob[:, c0:c0 + gw],
                                              in_=mm[:, 0:gw])
                    else:
                        nc.scalar.copy(out=ob[:, c0:c0 + gw], in_=mm[:, 0:gw])
                    if gi % 2 == 1:
                        oc, ow = GROUPS[gi - 1][0], GROUPS[gi - 1][1] + gw
                        nc.sync.dma_start(
                            OUT[j * P:(j + 1) * P, oc:oc + ow],
                            ob[:, oc:oc + ow])
                    if j == 0 and gi == 7:
                        # transposes for tiles 1,2 slot in here: late enough
                        # that the in-order tensor engine barely parks on
                        # v12, early enough that tile-1 matmuls are unblocked
                        emit_transpose(v12, 0, 1)
                        emit_transpose(v12, 1, 2)
                if j == 1:
                    # tile-3 head emitted after tile-1's vector casts; its
                    # transpose goes right before tile-3's matmuls
                    v3 = head(3, tbl123[:, 2:3, :], 1, nc.vector)
                if j == 2:
                    emit_transpose(v3, 0, 3)

    nc.compile()
    return nc


def _prep_inputs(inputs):
    x = np.asarray(inputs["x"])
    E0 = np.asarray(inputs["E0"], dtype=np.float32)
    E1 = np.asarray(inputs["E1"], dtype=np.float32)
    E2 = np.asarray(inputs["E2"], dtype=np.float32)
    E3 = np.asarray(inputs["E3"], dtype=np.float32)
    E4 = np.asarray(inputs["E4"], dtype=np.float32)
    E5 = np.asarray(inputs["E5"], dtype=np.float32)
    E6 = np.asarray(inputs["E6"], dtype=np.float32)
    rule_C = np.asarray(inputs["rule_C"], dtype=np.float32)
    rule_S = np.asarray(inputs["rule_S"], dtype=np.float32)
    has_rules = np.asarray(inputs["has_rules"])

    h, r, t = (x[:, 0].astype(np.int64), x[:, 1].astype(np.int64),
               x[:, 3].astype(np.int64))
    tb = t // CYCLE
    H = RANK // 2

    L = E0[h]
    R = E1[r]
    RC = rule_C[r]
    CT = E4[t]
    TM = E2[t] + E5[tb]
    TE = E3[t] + E6[tb]
    hr = has_rules[r].astype(np.float32)
    hs = hr * rule_S[r]

    def hsp(a):
        return a[:, :H], a[:, H:]

    L0, L1 = hsp(L)
    R0, R1 = hsp(R)
    RC0, RC1 = hsp(RC)

    C0, C1 = hsp(CT)
    tblex = np.concatenate([
        R0, R0, R1, -R1,          # RELX4
        RC0, RC1, -RC1, RC0,      # RCP
        C0, C0, C1, C1,           # CTdup
        TM, TE,
        L0, L1, -L1, -L0,         # E0GX
        np.repeat(hr[:, None], RANK, axis=1),
        hs[:, None] * R,
    ], axis=1).astype(np.float16)   # [B, TW]
    assert tblex.shape[1] == TW

    e0t = np.ascontiguousarray(E0.T.astype(np.float16))   # [128, 40000]

    tbl_by_bh = []
    for bh in range(BS):
        rows = tblex[bh * ROWS:(bh + 1) * ROWS]
        tbl_by_bh.append(np.ascontiguousarray(
            rows.reshape(NT, P, TW).transpose(1, 0, 2)))
    e0t_by_es = [np.ascontiguousarray(e0t[:, es * NSLICE:(es + 1) * NSLICE])
                 for es in range(ES)]

    ident = np.eye(P, dtype=np.float16)
    in_maps = []
    for c in range(NCORES):
        in_maps.append({
            "TBL": tbl_by_bh[c // ES],
            "E0T": e0t_by_es[c % ES],
            "IDN": ident,
        })
    return in_maps


def kernel(**inputs):
    from concourse.bass_utils import run_bass_kernel_spmd

    if "nc" not in _CACHE:
        _CACHE["nc"] = _build()
    nc = _CACHE["nc"]

    in_maps = _prep_inputs(inputs)
    res = run_bass_kernel_spmd(nc, in_maps, core_ids=list(range(NCORES)),
                               trace=TRACE)
    _CACHE["last_result"] = res
    out = np.empty((B, NENT), np.float32)
    for c in range(NCORES):
        bh, es = c // ES, c % ES
        out[bh * ROWS:(bh + 1) * ROWS,
            es * NSLICE:(es + 1) * NSLICE] = res.results[c]["OUT"]
    return out
